# revision 38
# baseline (speedup 1.0000x reference)
"""Dynamic depthwise-3x3 conv (AClayer) on 8 TRN2 NeuronCores.

Structure: out[n,ch,i,j] = sum_p w[n,ch,p] * xpad[n,ch,i+di(p),j+dj(p)]
where w[n,ch,:] = BN(conv1x1(avgpool16x16(x)))[n,:,ch//16,ch%16].

Sharding: core k = (sample k//2, row-half k%2), all 256 channels.
Two NEFF launches:
  Phase A: each core pools its shard via PE matmuls (conv-before-pool swap:
           sigma_raw[o,s] = sum_c w_conv[o,c] * sum_{8x8} x[c,...]) ->
           sigma_loc [9,128] f32 out. Two HW DMA rings (only the SP and ACT
           queues have hardware DGE; gpsimd's software queue is ~10x slower),
           512-col row-pair matmuls into 8 resident psum banks, per-pool-row
           DVE reduces + one combined reduce trailing the PE. wt rides at the
           head of chunk 0 (a separate [128,18] transfer stalls the ring with
           36-byte descriptors).
  Host:    relays the 8 tiny sigma blocks to every core (rotated so each
           core's own sample sits first). No math on host.
  Phase B: each core redundantly computes global BN stats (exact sync-BN;
           eps folded for the un-divided pooling sums), builds per-channel
           9-tap weights, and runs the stencil: channels on partitions,
           PE does rows [0,R_PE) via diagonal-matmul accumulation in PSUM
           (ACT evacuates), DVE does rows [R_PE,64) via tensor_scalar +
           tensor_tensor chains in bf16 with ACT producing 3-4 of the 9
           tap products. Odd-dj taps read X directly (odd-element offsets
           run at full DVE/ACT speed; no shifted copy needed). The BN chain
           is latency-tuned: ACT squares SGB in parallel with the S1 reduce,
           the subtract is folded into ACT's sqrt bias, and t comes from one
           scalar_tensor_tensor.

All DRAM inputs are host-packed chunk-major so every DMA reads a fully
contiguous HBM range (strided reads run at ~half bandwidth).
"""
from contextlib import ExitStack

import numpy as np
import ml_dtypes

import concourse.bass as bass
import concourse.mybir as mybir
from concourse.bass_utils import run_bass_kernel_spmd

bf16 = ml_dtypes.bfloat16
DT = mybir.dt
Alu = mybir.AluOpType
ActF = mybir.ActivationFunctionType

N_CORES = 8
CORE_IDS = list(range(N_CORES))

C, H, W = 256, 128, 128
RS, CS = 66, 130              # shard rows (with halo), padded cols
SH = RS * CS                  # 8580 elems per channel per shard
R_PE = 40                     # PE-region output rows per channel-block
R_DVE = 64 - R_PE             # 24
AROWS = R_PE + 2              # phase-B chunk A rows (covers PE region taps)
ALEN = AROWS * CS             # 5460
BLEN = SH - ALEN              # 3120
RD = R_DVE * 128              # 3072 free elems per DVE-region tile
EPS_EFF = 4096 * 1e-5         # eps for un-divided (x64) pooling sums
# phase-B PE: chunk c (4 output rows) of block cb accumulates in psum bank
# BANK[cb][c]; groups of <=4 chunks share one lhsT load per tap and are
# evacuated together so a chunk never waits on the previous group's bank
BANK = {0: [0, 1, 2, 3, 4, 5, 6, 7, 0, 1],
        1: [2, 3, 4, 5, 6, 7, 0, 1, 2, 3]}
GROUPS = [(0, (0, 1, 2, 3)), (0, (4, 5, 6, 7)), (0, (8, 9)),
          (1, (0, 1, 2, 3)), (1, (4, 5, 6, 7)), (1, (8, 9))]
# s_act threshold (evac'd chunk count) before group g may start: the group
# reuses banks first written 8 chunks earlier
ACT_WAIT = [0, 0, 4, 8, 10, 14]
# DVE-region taps: per cb, 9 taps (di,dj). DVE initializes the
# accumulator with tap (0,2) and does 4 more itself; ACT computes the
# other 4 products into TA and DVE adds them.
ACT_TAPS = [(0, 0), (0, 1), (1, 1), (2, 1)]
DVE_TAPS = [(1, 0), (1, 2), (2, 0), (2, 2)]


def build_phase_a():
    nc = bass.Bass()
    # x: 4 contiguous chunks (h0cb0, h0cb1, h1cb0, h1cb1), interior rows
    # 1..64 cols 1..128 only -> [128, 32*128] each
    # chunk 0 carries the 18 wt columns prepended per partition so the tiny
    # wt transfer doesn't stall the ring with 36-byte descriptors
    x0 = nc.declare_dram_parameter("x0", [128, 18 + 4096], DT.bfloat16,
                                   isOutput=False)
    x = nc.declare_dram_parameter("x", [3, 128, 4096], DT.bfloat16,
                                  isOutput=False)
    sig = nc.declare_dram_parameter("sig", [9, 128], DT.float32, isOutput=True)

    with (
        nc.sbuf_tensor("XA", [128, 18 + 4 * 4096], DT.bfloat16) as XA,
        nc.sbuf_tensor("R1", [9, 512], DT.float32) as R1,
        nc.sbuf_tensor("SIG", [9, 128], DT.float32) as SIG,
        nc.psum_tensor("PS", [9, 4096], DT.float32) as PS,
        nc.semaphore("s_a1") as s_a1,
        nc.semaphore("s_a2") as s_a2,
        nc.semaphore("s_mm") as s_mm,
        nc.semaphore("s_h") as s_h,
        nc.semaphore("s_red") as s_red,
        nc.semaphore("s_out") as s_out,
        nc.Block() as block,
    ):
        @block.sync
        def _(sync):
            # both HW DGE rings (SP + ACT) stream concurrently; each carries
            # one cb per half so PE can chase chunk arrivals
            sync.dma_start(out=XA[:, 0:4114], in_=x0[:, :]).then_inc(s_a1, 32)
            sync.dma_start(out=XA[:, 8210:12306], in_=x[1]).then_inc(s_a1, 16)
            sync.wait_ge(s_red, 2)
            sync.dma_start(out=sig[:, :], in_=SIG[:, :]).then_inc(s_out, 16)
            sync.wait_ge(s_out, 16)

        @block.scalar
        def _(sc):
            sc.dma_start(out=XA[:, 4114:8210], in_=x[0]).then_inc(s_a2, 16)
            sc.dma_start(out=XA[:, 12306:16402], in_=x[2]).then_inc(s_a2, 16)

        WT = XA[:, 0:18]

        @block.tensor
        def _(te):
            # warm-up on garbage SBUF data (no gate): bridges the idle
            # window until chunks land so the PE p-state is high
            for _ in range(16):
                te.matmul(PS[:, 3584:4096], lhsT=XA[:, 0:9],
                          rhs=XA[:, 18:530],
                          start=True, stop=True, skip_group_check=True)
            # pool-row pr (8 interior rows) accumulates its two row-quads
            # (x both cbs) into psum bank pr; all 8 pool-rows stay resident
            # so the PE never waits on a reduce
            for half in (0, 1):
                for cb in (0, 1):
                    if cb == 0:
                        te.wait_ge(s_a1, 32 + 16 * half)
                    else:
                        te.wait_ge(s_a2, 16 + 16 * half)
                    lhsT = WT[:, cb * 9: cb * 9 + 9]
                    coff = 18 + (2 * half + cb) * 4096
                    for q in range(8):
                        pr = 4 * half + q // 2
                        mm = te.matmul(
                            PS[:, pr * 512:(pr + 1) * 512], lhsT=lhsT,
                            rhs=XA[:, coff + q * 512: coff + q * 512 + 512],
                            start=(cb == 0 and q % 2 == 0),
                            stop=(cb == 1 and q % 2 == 1),
                            skip_group_check=True)
                        if cb == 1 and q % 2 == 1:
                            mm.then_inc(s_mm, 1)

        @block.vector
        def _(ve):
            # per pool-row: psum bank holds 4 row-pair sums x 16 pc x 8 dc;
            # reduce dc into an R1 slot (no self-hops: disjoint outputs),
            # then one combined row-reduce into SIG
            for pr in range(8):
                ve.wait_ge(s_mm, pr + 1)
                src = PS[:, pr * 512:(pr + 1) * 512].rearrange(
                    "o (r pc dc) -> o (r pc) dc", dc=8, pc=16)
                ve.tensor_reduce(R1[:, pr * 64:(pr + 1) * 64], src,
                                 axis=mybir.AxisListType.X,
                                 op=Alu.add).then_inc(s_h, 1)
            ve.wait_ge(s_h, 8)  # let the write tails commit
            s2 = R1[:, :].rearrange("o (pr r pc) -> o pr pc r", r=4, pc=16)
            sigv = SIG[:, :].rearrange("o (pr pc) -> o pr pc", pc=16)
            ve.tensor_reduce(sigv, s2, axis=mybir.AxisListType.X,
                             op=Alu.add).then_inc(s_red, 1)
            # self-sem hop so SIG's write tail is committed before the
            # out-DMA (sem'd via s_red) reads it
            ve.wait_ge(s_red, 1)
            ve.nop().then_inc(s_red, 1)

    return nc


def build_phase_b():
    nc = bass.Bass()
    xa = nc.declare_dram_parameter("xa", [2, 128, ALEN], DT.bfloat16,
                                   isOutput=False)
    xb = nc.declare_dram_parameter("xb", [2, 128, BLEN], DT.bfloat16,
                                   isOutput=False)
    sgb = nc.declare_dram_parameter("sgb", [9, 1026], DT.bfloat16,
                                    isOutput=False)
    out0 = nc.declare_dram_parameter("out0", [128, 8192], DT.bfloat16,
                                     isOutput=True)
    out1 = nc.declare_dram_parameter("out1", [128, 8192], DT.bfloat16,
                                     isOutput=True)

    with ExitStack() as ctx:
        e = ctx.enter_context
        X = e(nc.sbuf_tensor("X", [128, 2 * SH], DT.bfloat16))
        OA = e(nc.sbuf_tensor("OA", [128, 2 * 8192], DT.bfloat16))
        ACC = e(nc.sbuf_tensor("ACC", [128, RD], DT.bfloat16))
        TMP = e(nc.sbuf_tensor("TMP", [128, RD], DT.bfloat16))
        TA = e(nc.sbuf_tensor("TA", [128, 8 * RD], DT.bfloat16))
        SGB = e(nc.sbuf_tensor("SGB", [9, 1026], DT.bfloat16))
        SQ = e(nc.sbuf_tensor("SQ", [9, 1024], DT.float32))
        ST = e(nc.sbuf_tensor("ST", [9, 12], DT.float32))
        WSM = e(nc.sbuf_tensor("WSM", [9, 256], DT.float32))
        WCH = e(nc.sbuf_tensor("WCH", [128, 18], DT.float32))
        DIAG = e(nc.sbuf_tensor("DIAG", [128, 18 * 128], DT.bfloat16))
        EYE128 = e(nc.sbuf_tensor("EYE128", [128, 128], DT.bfloat16))
        EYE9 = e(nc.sbuf_tensor("EYE9", [9, 9], DT.float32))
        PT = e(nc.psum_tensor("PT", [128, 4096], DT.float32))
        s_in1 = e(nc.semaphore("s_in1"))
        s_eye = e(nc.semaphore("s_eye"))
        s_sq = e(nc.semaphore("s_sq"))
        s_in2 = e(nc.semaphore("s_in2"))
        s_bn1 = e(nc.semaphore("s_bn1"))
        s_act1 = e(nc.semaphore("s_act1"))
        s_bn2 = e(nc.semaphore("s_bn2"))
        s_tp = e(nc.semaphore("s_tp"))
        s_wc = e(nc.semaphore("s_wc"))
        s_wch = e(nc.semaphore("s_wch"))
        s_wch2 = e(nc.semaphore("s_wch2"))
        s_pe = e(nc.semaphore("s_pe"))
        s_act = e(nc.semaphore("s_act"))
        s_ta = e(nc.semaphore("s_ta"))
        s_dve = e(nc.semaphore("s_dve"))
        s_out = e(nc.semaphore("s_out"))
        s_v = e(nc.semaphore("s_v"))
        s_a = e(nc.semaphore("s_a"))
        block = e(nc.Block())

        def xcb_view(cb):
            return X[:, cb * SH:(cb + 1) * SH].rearrange(
                "p (r c) -> p r c", c=CS)

        @block.sync
        def _(sync):
            # both HW rings share ~345GB/s; order pieces by need time and
            # split xa0 across rings so cb0's PE inputs land first
            sync.dma_start(out=SGB[:, :], in_=sgb[:, :]).then_inc(s_in1, 16)
            sync.dma_start(out=X[:, 0:2730], in_=xa[0][:, 0:2730]).then_inc(
                s_in1, 16)
            sync.dma_start(out=X[:, SH:SH + ALEN], in_=xa[1]).then_inc(
                s_in1, 16)
            # cb0 output drains (PE chunks 0-7, 8-9, then the DVE region)
            sync.wait_ge(s_act, 8)
            sync.dma_start(out=out0[:, 0:4096],
                           in_=OA[:, 0:4096]).then_inc(s_out, 16)
            sync.wait_ge(s_act, 10)
            sync.dma_start(out=out0[:, 4096:5120],
                           in_=OA[:, 4096:5120]).then_inc(s_out, 16)
            hh = R_DVE // 2 * 128
            sync.wait_ge(s_dve, 1)
            sync.dma_start(out=out0[:, 5120:5120 + hh],
                           in_=OA[:, 5120:5120 + hh]).then_inc(s_out, 16)
            sync.wait_ge(s_dve, 2)
            sync.dma_start(out=out0[:, 5120 + hh:8192],
                           in_=OA[:, 5120 + hh:8192]).then_inc(s_out, 16)
            sync.wait_ge(s_dve, 3)
            sync.dma_start(out=out1[:, 5120:5120 + hh],
                           in_=OA[:, 13312:13312 + hh]).then_inc(s_out, 16)
            sync.wait_ge(s_dve, 4)
            sync.dma_start(out=out1[:, 5120 + hh:8192],
                           in_=OA[:, 13312 + hh:16384]).then_inc(s_out, 16)
            sync.wait_ge(s_out, 144)

        @block.gpsimd
        def _(gp):
            # memset -> affine_select is a same-engine RAW on tiny ops:
            # self-semaphore the hop
            gp.memset(EYE128[:, :], 0.0)
            gp.memset(EYE9[:, :], 0.0).then_inc(s_eye, 1)
            gp.wait_ge(s_eye, 1)
            gp.affine_select(out=EYE128[:, :], in_=EYE128[:, :],
                             compare_op=Alu.not_equal, fill=1.0, base=0,
                             pattern=[[-1, 128]], channel_multiplier=1)
            gp.affine_select(out=EYE9[:, :], in_=EYE9[:, :],
                             compare_op=Alu.not_equal, fill=1.0, base=0,
                             pattern=[[-1, 9]], channel_multiplier=1)
            gp.nop().then_inc(s_eye, 1)

        @block.scalar
        def _(sc):
            # second HW DGE ring
            sc.dma_start(out=X[:, 2730:ALEN],
                         in_=xa[0][:, 2730:ALEN]).then_inc(s_in2, 16)
            sc.dma_start(out=X[:, ALEN:SH], in_=xb[0]).then_inc(s_in2, 16)
            sc.dma_start(out=X[:, SH + ALEN:2 * SH],
                         in_=xb[1]).then_inc(s_in2, 16)
            # dummy sqrt to trigger the ACT table load early
            sc.activation(ST[:, 7:8], ST[:, 6:7], ActF.Sqrt)
            sc.wait_ge(s_in1, 16)
            sc.activation(SQ[:, :], SGB[:, 0:1024], ActF.Square,
                          accum_out=ST[:, 1:2]).then_inc(s_sq, 1)
            sc.wait_ge(s_bn1, 1)
            # ST7 = sqrt(-S1^2 + ST3); self-sem hop commits the write
            sc.activation(ST[:, 7:8], ST[:, 2:3], ActF.Sqrt, scale=-1.0,
                          bias=ST[:, 3:4]).then_inc(s_a, 1)
            sc.wait_ge(s_a, 1)
            sc.nop().then_inc(s_act1, 1)

            na = [1]

            def evac(pt_ranges, oa_off, n_chunks, g):
                sc.wait_ge(s_pe, g + 1)
                off = oa_off
                for lo, hi in pt_ranges:
                    sc.activation(OA[:, off: off + (hi - lo)],
                                  PT[:, lo:hi], ActF.Copy).then_inc(s_a, 1)
                    off += hi - lo
                    na[0] += 1
                # commit hop so the out-DMA reads a completed OA write tail
                sc.wait_ge(s_a, na[0])
                sc.nop().then_inc(s_act, n_chunks)

            def product(cb, di, dj, dst, init=False):
                tap = xcb_view(cb)[:, R_PE + di: R_PE + di + R_DVE,
                                   dj: dj + 128]
                sc.activation(dst.rearrange("p (r c) -> p r c", c=128), tap,
                              ActF.Copy,
                              scale=WCH[:, cb * 9 + 3 * di + dj:
                                        cb * 9 + 3 * di + dj + 1]
                              ).then_inc(s_ta, 1)

            # cb0 tap products into TA
            sc.wait_ge(s_wc, 1)
            sc.wait_ge(s_in1, 32)
            sc.wait_ge(s_in2, 32)
            for j, (di, dj) in enumerate(ACT_TAPS):
                product(0, di, dj, TA[:, j * RD:(j + 1) * RD])
            # cb1 DIAG builds (DVE builds cb0's)
            sc.wait_ge(s_eye, 2)
            for p in range(9):
                i = 9 + p
                sc.activation(DIAG[:, i * 128:(i + 1) * 128], EYE128[:, :],
                              ActF.Copy,
                              scale=WCH[:, i:i + 1]).then_inc(s_wch2, 1)
            evac([(0, 2048)], 0, 4, 0)           # A0: banks 0-3
            evac([(2048, 4096)], 2048, 4, 1)     # A1: banks 4-7
            evac([(0, 1024)], 4096, 2, 2)        # A2: banks 0-1
            # cb1 tap products
            sc.wait_ge(s_in1, 48)
            sc.wait_ge(s_in2, 48)
            for j, (di, dj) in enumerate(ACT_TAPS):
                product(1, di, dj, TA[:, (4 + j) * RD:(5 + j) * RD])
            evac([(1024, 3072)], 8192, 4, 3)     # B0: banks 2-5
            sc.dma_start(out=out1[:, 0:2048],
                         in_=OA[:, 8192:10240]).then_inc(s_out, 16)
            evac([(3072, 4096), (0, 1024)], 10240, 4, 4)  # B1: banks 6701
            sc.dma_start(out=out1[:, 2048:4096],
                         in_=OA[:, 10240:12288]).then_inc(s_out, 16)
            evac([(1024, 2048)], 12288, 2, 5)    # B2: banks 2-3
            sc.dma_start(out=out1[:, 4096:5120],
                         in_=OA[:, 12288:13312]).then_inc(s_out, 16)

        @block.vector
        def _(ve):
            # back-to-back dependent DVE ops on tiny operands race (the next
            # op's reads overlap the previous op's in-flight writes), so the
            # whole BN small-op chain is self-semaphored hop by hop.
            vc = [0]

            def step(ins):
                vc[0] += 1
                ins.then_inc(s_v, 1)
                ve.wait_ge(s_v, vc[0])

            # BN statistics in the raw-sums basis, minimal serial depth:
            #   Dv = 1024*S2 - S1^2 = 1024^2 * var_raw
            #   s  = 1024*gamma / sqrt(Dv + 1024^2*eps)
            #   t  = beta - (S1/1024)*s
            ve.wait_ge(s_in1, 16)
            ve.tensor_scalar(ST[:, 10:11], SGB[:, 1024:1025], 1024.0, None,
                             Alu.mult)                         # gamma*1024
            step(ve.tensor_reduce(ST[:, 0:1], SGB[:, 0:1024],
                                  axis=mybir.AxisListType.X, op=Alu.add))
            ve.tensor_scalar(ST[:, 2:3], ST[:, 0:1], ST[:, 0:1], None,
                             Alu.mult)                         # S1^2
            ve.tensor_scalar(ST[:, 4:5], ST[:, 0:1], -1.0 / 1024, None,
                             Alu.mult)                         # -S1/1024
            ve.wait_ge(s_sq, 1)   # ACT's square also accumulated S2
            step(ve.tensor_scalar(ST[:, 3:4], ST[:, 1:2], 1024.0,
                                  1048576.0 * EPS_EFF, Alu.mult, Alu.add))
            ve.nop().then_inc(s_bn1, 1)
            # ACT computes ST7 = sqrt(ST3 - S1^2) with the subtract fused
            ve.wait_ge(s_act1, 1)
            step(ve.reciprocal(ST[:, 8:9], ST[:, 7:8]))
            step(ve.tensor_scalar(ST[:, 9:10], ST[:, 10:11], ST[:, 8:9],
                                  None, Alu.mult))             # s
            step(ve.scalar_tensor_tensor(ST[:, 11:12], ST[:, 4:5],
                                         ST[:, 9:10], SGB[:, 1025:1026],
                                         Alu.mult, Alu.add))   # t
            step(ve.tensor_scalar(WSM[:, :], SGB[:, 0:256], ST[:, 9:10],
                                  ST[:, 11:12], Alu.mult, Alu.add))
            ve.nop().then_inc(s_bn2, 1)
            ve.wait_ge(s_tp, 1)
            ve.tensor_copy(WCH[:, 0:9], PT[:, 0:9])
            step(ve.tensor_copy(WCH[:, 9:18], PT[:, 9:18]))
            ve.nop().then_inc(s_wc, 1)
            ve.wait_ge(s_eye, 2)
            for p in range(9):  # cb0 DIAGs (ACT builds cb1's)
                ve.tensor_scalar(DIAG[:, p * 128:(p + 1) * 128],
                                 EYE128[:, :], WCH[:, p:p + 1], None,
                                 Alu.mult).then_inc(s_wch, 1)
            # DVE stencil region: rows [R_PE, 64) of each channel block
            for cb in (0, 1):
                accv = ACC[:, :].rearrange("p (r c) -> p r c", c=128)
                tmpv = TMP[:, :].rearrange("p (r c) -> p r c", c=128)
                outv = OA[:, cb * 8192 + R_PE * 128: cb * 8192 + 8192]
                outv = outv.rearrange("p (r c) -> p r c", c=128)
                xcb = xcb_view(cb)
                if cb == 0:
                    ve.wait_ge(s_in1, 32)
                    ve.wait_ge(s_in2, 32)
                else:
                    ve.wait_ge(s_in1, 48)
                    ve.wait_ge(s_in2, 48)
                taps = [(0, 2)] + DVE_TAPS  # first tap initializes ACC
                for i, (di, dj) in enumerate(taps):
                    tap = xcb[:, R_PE + di: R_PE + di + R_DVE, dj: dj + 128]
                    wsc = WCH[:, cb * 9 + 3 * di + dj:
                              cb * 9 + 3 * di + dj + 1]
                    if i == 0:
                        ve.tensor_scalar(accv, tap, wsc, None, Alu.mult)
                    else:
                        ve.tensor_scalar(tmpv, tap, wsc, None, Alu.mult)
                        ve.tensor_tensor(accv, tmpv, accv, Alu.add)
                for j in range(4):  # ACT-produced tap products
                    ve.wait_ge(s_ta, 4 * cb + j + 1)
                    tav = TA[:, (4 * cb + j) * RD: (4 * cb + j + 1) * RD]
                    tav = tav.rearrange("p (r c) -> p r c", c=128)
                    if j < 3:
                        last = ve.tensor_tensor(accv, tav, accv, Alu.add)
                    else:
                        # final tap writes OA in halves so the drain of the
                        # first half overlaps the second
                        h = R_DVE // 2
                        step(ve.tensor_tensor(outv[:, 0:h], tav[:, 0:h],
                                              accv[:, 0:h], Alu.add))
                        ve.nop().then_inc(s_dve, 1)
                        last = ve.tensor_tensor(outv[:, h:R_DVE],
                                                tav[:, h:R_DVE],
                                                accv[:, h:R_DVE], Alu.add)
                # self-sem hop so the OA write tail is committed before
                # the out-DMA (sem'd via s_dve) reads it
                step(last)
                ve.nop().then_inc(s_dve, 1)

        @block.tensor
        def _(te):
            te.wait_ge(s_eye, 2)
            # HAM warm-up bursts: keep PE busy (never >3.4us idle) through
            # the BN-weights chain without blocking real work for long
            for _ in range(11):
                te.matmul(PT[:, 3584:4096], lhsT=EYE128[:, :],
                          rhs=X[:, 0:512], start=True, stop=True)
            te.wait_ge(s_bn2, 1)
            # both transposes land in psum bank 0: the second must not
            # re-clear the bank (start=True wipes the whole bank)
            te.matmul(PT[:, 0:9], lhsT=WSM[:, 0:128], rhs=EYE9[:, :],
                      is_transpose=True, start=True, stop=False,
                      skip_group_check=True)
            te.matmul(PT[:, 9:18], lhsT=WSM[:, 128:256], rhs=EYE9[:, :],
                      is_transpose=True, start=False, stop=True,
                      skip_group_check=True).then_inc(s_tp, 1)
            for _ in range(2):  # stay warm through the diag builds
                te.matmul(PT[:, 3584:4096], lhsT=EYE128[:, :],
                          rhs=X[:, 0:512], start=True, stop=True)
            te.wait_ge(s_in1, 32)  # xa0 loaded
            te.wait_ge(s_in2, 16)
            for g, (cb, grp) in enumerate(GROUPS):
                if cb == 1 and grp[0] == 0:
                    te.wait_ge(s_in1, 48)   # xa1 loaded
                    te.wait_ge(s_wch2, 9)   # cb1 DIAGs built
                if ACT_WAIT[g]:
                    te.wait_ge(s_act, ACT_WAIT[g])
                xcb = xcb_view(cb)
                for p in range(9):
                    if g == 0:
                        te.wait_ge(s_wch, p + 1)
                    di, dj = p // 3, p % 3
                    lhsT = DIAG[:, (cb * 9 + p) * 128:
                                (cb * 9 + p) * 128 + 128]
                    for c in grp:
                        rhs = xcb[:, 4 * c + di: 4 * c + di + 4,
                                  dj: dj + 128]
                        mm = te.matmul(
                            PT[:, BANK[cb][c] * 512:
                               BANK[cb][c] * 512 + 512],
                            lhsT=lhsT, rhs=rhs,
                            start=(p == 0), stop=(p == 8))
                mm.then_inc(s_pe, 1)

    return nc


def host_prep(x, w_conv):
    """Shard + pack all phase inputs (layout only, no math)."""
    n = x.shape[0]
    xpad = np.zeros((n, C, H + 2, W + 2), np.float32)
    xpad[:, :, 1:-1, 1:-1] = x
    xbf = xpad.astype(bf16)
    wt = np.ascontiguousarray(
        w_conv.reshape(9, 2, 128).transpose(2, 1, 0).reshape(128, 18)
    ).astype(bf16)
    maps_a, maps_b = [], []
    for k in range(N_CORES):
        sh = xbf[k // 2, :, 64 * (k % 2):64 * (k % 2) + 66, :]  # (256,66,130)
        shv = np.ascontiguousarray(sh).reshape(2, 128, RS, CS)
        # phase A: interior rows 1..64, cols 1..128, chunk-major (half, cb)
        xi = shv[:, :, 1:65, 1:129]                 # (2,128,64,128)
        xa_in = np.ascontiguousarray(
            xi.reshape(2, 128, 2, 32 * 128).transpose(2, 0, 1, 3)
        ).reshape(4, 128, 4096)
        x0_in = np.ascontiguousarray(
            np.concatenate([wt, xa_in[0]], axis=1))
        maps_a.append({"x0": x0_in, "x": xa_in[1:4]})
        # phase B: per-cb contiguous chunks
        flat = shv.reshape(2, 128, SH)
        xa_b = np.ascontiguousarray(flat[:, :, 0:ALEN])
        xb_b = np.ascontiguousarray(flat[:, :, ALEN:SH])
        maps_b.append({"xa": xa_b, "xb": xb_b})
    return maps_a, maps_b


def sgb_for_cores(sig, gamma, beta):
    """sig: [8, 9, 128] raw per-core sigma -> per-core sgb arrays."""
    sig_all = sig.reshape(4, 2, 9, 128).transpose(0, 2, 1, 3).reshape(4, 9, 256)
    out = []
    for k in range(N_CORES):
        ni = k // 2
        order = [ni] + [j for j in range(4) if j != ni]
        sgb = np.zeros((9, 1026), np.float32)
        sgb[:, 0:1024] = sig_all[order].transpose(1, 0, 2).reshape(9, 1024)
        sgb[:, 1024] = gamma
        sgb[:, 1025] = beta
        out.append(sgb.astype(bf16))
    return out


def assemble_output(res_b, n):
    outf = np.empty((n, C, H, W), np.float32)
    for k in range(N_CORES):
        r = res_b.results[k]
        ni, r0 = k // 2, 64 * (k % 2)
        for cb, name in enumerate(("out0", "out1")):
            chs = slice(cb * 128, cb * 128 + 128)
            outf[ni, chs, r0:r0 + 64, :] = \
                np.asarray(r[name]).reshape(128, 64, W).astype(np.float32)
    return outf


_CACHE = {}


def kernel(x, w_conv, gamma, beta):
    x = np.asarray(x, dtype=np.float32)
    w_conv = np.asarray(w_conv, dtype=np.float32)
    gamma = np.asarray(gamma, dtype=np.float32)
    beta = np.asarray(beta, dtype=np.float32)

    if "A" not in _CACHE:
        _CACHE["A"] = build_phase_a()
        _CACHE["B"] = build_phase_b()

    maps_a, maps_b = host_prep(x, w_conv)
    res_a = run_bass_kernel_spmd(_CACHE["A"], maps_a, CORE_IDS)
    sig = np.stack([np.asarray(res_a.results[k]["sig"]) for k in CORE_IDS])
    sgbs = sgb_for_cores(sig, gamma, beta)
    for m, sgb in zip(maps_b, sgbs):
        m["sgb"] = sgb
    res_b = run_bass_kernel_spmd(_CACHE["B"], maps_b, CORE_IDS)
    return assemble_output(res_b, x.shape[0])


# revision 39
# speedup vs baseline: 1.1538x; 1.1538x over previous
"""Dynamic depthwise-3x3 conv (AClayer) on 8 TRN2 NeuronCores.

Structure: out[n,ch,i,j] = sum_p w[n,ch,p] * xpad[n,ch,i+di(p),j+dj(p)]
where w[n,ch,:] = BN(conv1x1(avgpool16x16(x)))[n,:,ch//16,ch%16].

Sharding: core k = (sample k//2, row-half k%2), all 256 channels.
Two NEFF launches:
  Phase A: each core pools its shard via PE matmuls (conv-before-pool swap:
           sigma_raw[o,s] = sum_c w_conv[o,c] * sum_{8x8} x[c,...]) ->
           sigma_loc [9,128] f32 out. Two HW DMA rings (only the SP and ACT
           queues have hardware DGE; gpsimd's software queue is ~10x slower),
           512-col row-pair matmuls into 8 resident psum banks, per-pool-row
           DVE reduces + one combined reduce trailing the PE. wt rides at the
           head of chunk 0 (a separate [128,18] transfer stalls the ring with
           36-byte descriptors).
  Host:    relays the 8 tiny sigma blocks to every core (rotated so each
           core's own sample sits first). No math on host.
  Phase B: each core redundantly computes global BN stats (exact sync-BN;
           eps folded for the un-divided pooling sums), builds per-channel
           9-tap weights, and runs the stencil: channels on partitions,
           PE does rows [0,R_PE) via diagonal-matmul accumulation in PSUM
           (ACT evacuates), DVE does rows [R_PE,64) via tensor_scalar +
           tensor_tensor chains in bf16 with ACT producing 3-4 of the 9
           tap products. Odd-dj taps read X directly (odd-element offsets
           run at full DVE/ACT speed; no shifted copy needed). The BN chain
           is latency-tuned: ACT squares SGB in parallel with the S1 reduce,
           the subtract is folded into ACT's sqrt bias, and t comes from one
           scalar_tensor_tensor.

All DRAM inputs are host-packed chunk-major so every DMA reads a fully
contiguous HBM range (strided reads run at ~half bandwidth).
"""
from contextlib import ExitStack

import numpy as np
import ml_dtypes

import concourse.bass as bass
import concourse.mybir as mybir
from concourse.bass_utils import run_bass_kernel_spmd

bf16 = ml_dtypes.bfloat16
DT = mybir.dt
Alu = mybir.AluOpType
ActF = mybir.ActivationFunctionType

N_CORES = 8
CORE_IDS = list(range(N_CORES))

C, H, W = 256, 128, 128
RS, CS = 66, 130              # shard rows (with halo), padded cols
SH = RS * CS                  # 8580 elems per channel per shard
R_PE = 40                     # PE-region output rows per channel-block
R_DVE = 64 - R_PE             # 24
AROWS = R_PE + 2              # phase-B chunk A rows (covers PE region taps)
ALEN = AROWS * CS             # 5460
BLEN = SH - ALEN              # 3120
RD = R_DVE * 128              # 3072 free elems per DVE-region tile
EPS_EFF = 4096 * 1e-5         # eps for un-divided (x64) pooling sums
# phase-B PE: chunk c (4 output rows) of block cb accumulates in psum bank
# BANK[cb][c]; groups of <=4 chunks share one lhsT load per tap and are
# evacuated together so a chunk never waits on the previous group's bank
BANK = {0: [0, 1, 2, 3, 4, 5, 6, 7, 0, 1],
        1: [2, 3, 4, 5, 6, 7, 0, 1, 2, 3]}
GROUPS = [(0, (0, 1, 2, 3)), (0, (4, 5, 6, 7)), (0, (8, 9)),
          (1, (0, 1, 2, 3)), (1, (4, 5, 6, 7)), (1, (8,)), (1, (9,))]
# s_act threshold (evac'd chunk count) before group g may start: the group
# reuses banks first written 8 chunks earlier
ACT_WAIT = [0, 0, 4, 8, 10, 14, 14]
# DVE-region taps: per cb, 9 taps (di,dj). DVE initializes the
# accumulator with tap (0,2) and does 4 more itself; ACT computes the
# other 4 products into TA and DVE adds them.
ACT_TAPS = [(0, 0), (0, 1), (1, 1), (2, 1)]
DVE_TAPS = [(1, 0), (1, 2), (2, 0), (2, 2)]


def build_phase_a():
    nc = bass.Bass()
    # x: 4 contiguous chunks (h0cb0, h0cb1, h1cb0, h1cb1), interior rows
    # 1..64 cols 1..128 only -> [128, 32*128] each
    # chunk 0 carries the 18 wt columns prepended per partition so the tiny
    # wt transfer doesn't stall the ring with 36-byte descriptors
    x0 = nc.declare_dram_parameter("x0", [128, 18 + 4096], DT.bfloat16,
                                   isOutput=False)
    x = nc.declare_dram_parameter("x", [3, 128, 4096], DT.bfloat16,
                                  isOutput=False)
    sig = nc.declare_dram_parameter("sig", [9, 128], DT.float32, isOutput=True)

    with (
        nc.sbuf_tensor("XA", [128, 18 + 4 * 4096], DT.bfloat16) as XA,
        nc.sbuf_tensor("R1", [9, 512], DT.float32) as R1,
        nc.sbuf_tensor("SIG", [9, 128], DT.float32) as SIG,
        nc.psum_tensor("PS", [9, 4096], DT.float32) as PS,
        nc.semaphore("s_a1") as s_a1,
        nc.semaphore("s_a2") as s_a2,
        nc.semaphore("s_mm") as s_mm,
        nc.semaphore("s_h") as s_h,
        nc.semaphore("s_red") as s_red,
        nc.semaphore("s_out") as s_out,
        nc.Block() as block,
    ):
        @block.sync
        def _(sync):
            # both HW DGE rings (SP + ACT) stream concurrently; each carries
            # one cb per half so PE can chase chunk arrivals
            sync.dma_start(out=XA[:, 0:4114], in_=x0[:, :]).then_inc(s_a1, 32)
            sync.dma_start(out=XA[:, 8210:12306], in_=x[1]).then_inc(s_a1, 16)
            sync.wait_ge(s_red, 3)
            sync.dma_start(out=sig[:, :], in_=SIG[:, :]).then_inc(s_out, 16)
            sync.wait_ge(s_out, 16)

        @block.scalar
        def _(sc):
            sc.dma_start(out=XA[:, 4114:8210], in_=x[0]).then_inc(s_a2, 16)
            sc.dma_start(out=XA[:, 12306:16402], in_=x[2]).then_inc(s_a2, 16)

        WT = XA[:, 0:18]

        @block.tensor
        def _(te):
            # warm-up on garbage SBUF data (no gate): bridges the idle
            # window until chunks land so the PE p-state is high
            for _ in range(16):
                te.matmul(PS[:, 3584:4096], lhsT=XA[:, 0:9],
                          rhs=XA[:, 18:530],
                          start=True, stop=True, skip_group_check=True)
            # pool-row pr (8 interior rows) accumulates its two row-quads
            # (x both cbs) into psum bank pr; all 8 pool-rows stay resident
            # so the PE never waits on a reduce
            for half in (0, 1):
                for cb in (0, 1):
                    if cb == 0:
                        te.wait_ge(s_a1, 32 + 16 * half)
                    else:
                        te.wait_ge(s_a2, 16 + 16 * half)
                    lhsT = WT[:, cb * 9: cb * 9 + 9]
                    coff = 18 + (2 * half + cb) * 4096
                    for q in range(8):
                        pr = 4 * half + q // 2
                        mm = te.matmul(
                            PS[:, pr * 512:(pr + 1) * 512], lhsT=lhsT,
                            rhs=XA[:, coff + q * 512: coff + q * 512 + 512],
                            start=(cb == 0 and q % 2 == 0),
                            stop=(cb == 1 and q % 2 == 1),
                            skip_group_check=True)
                        if cb == 1 and q % 2 == 1:
                            mm.then_inc(s_mm, 1)

        @block.vector
        def _(ve):
            # per pool-row: psum bank holds 4 row-pair sums x 16 pc x 8 dc;
            # reduce dc into an R1 slot (no self-hops: disjoint outputs),
            # then one combined row-reduce into SIG
            for pr in range(8):
                ve.wait_ge(s_mm, pr + 1)
                src = PS[:, pr * 512:(pr + 1) * 512].rearrange(
                    "o (r pc dc) -> o (r pc) dc", dc=8, pc=16)
                ve.tensor_reduce(R1[:, pr * 64:(pr + 1) * 64], src,
                                 axis=mybir.AxisListType.X,
                                 op=Alu.add).then_inc(s_h, 1)
            for hf in (0, 1):
                ve.wait_ge(s_h, 4 + 4 * hf)  # slot write tails committed
                s2 = R1[:, hf * 256:(hf + 1) * 256].rearrange(
                    "o (pr r pc) -> o pr pc r", r=4, pc=16)
                sigv = SIG[:, hf * 64:(hf + 1) * 64].rearrange(
                    "o (pr pc) -> o pr pc", pc=16)
                ve.tensor_reduce(sigv, s2, axis=mybir.AxisListType.X,
                                 op=Alu.add).then_inc(s_red, 1)
            # self-sem hop so SIG's write tail is committed before the
            # out-DMA (sem'd via s_red) reads it
            ve.wait_ge(s_red, 2)
            ve.nop().then_inc(s_red, 1)

    return nc


def build_phase_b():
    nc = bass.Bass()
    xa = nc.declare_dram_parameter("xa", [2, 128, ALEN], DT.bfloat16,
                                   isOutput=False)
    xb = nc.declare_dram_parameter("xb", [2, 128, BLEN], DT.bfloat16,
                                   isOutput=False)
    sgb = nc.declare_dram_parameter("sgb", [9, 1026], DT.bfloat16,
                                    isOutput=False)
    out0 = nc.declare_dram_parameter("out0", [128, 8192], DT.bfloat16,
                                     isOutput=True)
    out1 = nc.declare_dram_parameter("out1", [128, 8192], DT.bfloat16,
                                     isOutput=True)

    with ExitStack() as ctx:
        e = ctx.enter_context
        X = e(nc.sbuf_tensor("X", [128, 2 * SH], DT.bfloat16))
        OA = e(nc.sbuf_tensor("OA", [128, 2 * 8192], DT.bfloat16))
        ACC = e(nc.sbuf_tensor("ACC", [128, RD], DT.bfloat16))
        TMP = e(nc.sbuf_tensor("TMP", [128, RD], DT.bfloat16))
        TA = e(nc.sbuf_tensor("TA", [128, 8 * RD], DT.bfloat16))
        SGB = e(nc.sbuf_tensor("SGB", [9, 1026], DT.bfloat16))
        SQ = e(nc.sbuf_tensor("SQ", [9, 1024], DT.float32))
        ST = e(nc.sbuf_tensor("ST", [9, 12], DT.float32))
        WSM = e(nc.sbuf_tensor("WSM", [9, 256], DT.float32))
        WCH = e(nc.sbuf_tensor("WCH", [128, 18], DT.float32))
        DIAG = e(nc.sbuf_tensor("DIAG", [128, 18 * 128], DT.bfloat16))
        EYE128 = e(nc.sbuf_tensor("EYE128", [128, 128], DT.bfloat16))
        EYE9 = e(nc.sbuf_tensor("EYE9", [9, 9], DT.float32))
        PT = e(nc.psum_tensor("PT", [128, 4096], DT.float32))
        s_in1 = e(nc.semaphore("s_in1"))
        s_eye = e(nc.semaphore("s_eye"))
        s_sq = e(nc.semaphore("s_sq"))
        s_in2 = e(nc.semaphore("s_in2"))
        s_bn1 = e(nc.semaphore("s_bn1"))
        s_act1 = e(nc.semaphore("s_act1"))
        s_bn2 = e(nc.semaphore("s_bn2"))
        s_tp = e(nc.semaphore("s_tp"))
        s_wc = e(nc.semaphore("s_wc"))
        s_wch = e(nc.semaphore("s_wch"))
        s_wch2 = e(nc.semaphore("s_wch2"))
        s_pe = e(nc.semaphore("s_pe"))
        s_act = e(nc.semaphore("s_act"))
        s_ta = e(nc.semaphore("s_ta"))
        s_dve = e(nc.semaphore("s_dve"))
        s_out = e(nc.semaphore("s_out"))
        s_v = e(nc.semaphore("s_v"))
        s_a = e(nc.semaphore("s_a"))
        block = e(nc.Block())

        def xcb_view(cb):
            return X[:, cb * SH:(cb + 1) * SH].rearrange(
                "p (r c) -> p r c", c=CS)

        @block.sync
        def _(sync):
            # both HW rings share ~345GB/s; order pieces by need time and
            # split xa0 across rings so cb0's PE inputs land first
            sync.dma_start(out=SGB[:, :], in_=sgb[:, :]).then_inc(s_in1, 16)
            sync.dma_start(out=X[:, 0:2730], in_=xa[0][:, 0:2730]).then_inc(
                s_in1, 16)
            sync.dma_start(out=X[:, SH:SH + ALEN], in_=xa[1]).then_inc(
                s_in1, 16)
            # cb0 output drains (PE chunks 0-7, 8-9, then the DVE region)
            sync.wait_ge(s_act, 8)
            sync.dma_start(out=out0[:, 0:4096],
                           in_=OA[:, 0:4096]).then_inc(s_out, 16)
            sync.wait_ge(s_act, 10)
            sync.dma_start(out=out0[:, 4096:5120],
                           in_=OA[:, 4096:5120]).then_inc(s_out, 16)
            hh = R_DVE // 2 * 128
            sync.wait_ge(s_dve, 1)
            sync.dma_start(out=out0[:, 5120:5120 + hh],
                           in_=OA[:, 5120:5120 + hh]).then_inc(s_out, 16)
            sync.wait_ge(s_dve, 2)
            sync.dma_start(out=out0[:, 5120 + hh:8192],
                           in_=OA[:, 5120 + hh:8192]).then_inc(s_out, 16)
            sync.wait_ge(s_dve, 3)
            sync.dma_start(out=out1[:, 5120:5120 + hh],
                           in_=OA[:, 13312:13312 + hh]).then_inc(s_out, 16)
            sync.wait_ge(s_dve, 4)
            sync.dma_start(out=out1[:, 5120 + hh:8192],
                           in_=OA[:, 13312 + hh:16384]).then_inc(s_out, 16)
            sync.wait_ge(s_out, 144)

        @block.gpsimd
        def _(gp):
            # memset -> affine_select is a same-engine RAW on tiny ops:
            # self-semaphore the hop
            gp.memset(EYE128[:, :], 0.0)
            gp.memset(EYE9[:, :], 0.0).then_inc(s_eye, 1)
            gp.wait_ge(s_eye, 1)
            gp.affine_select(out=EYE128[:, :], in_=EYE128[:, :],
                             compare_op=Alu.not_equal, fill=1.0, base=0,
                             pattern=[[-1, 128]], channel_multiplier=1)
            gp.affine_select(out=EYE9[:, :], in_=EYE9[:, :],
                             compare_op=Alu.not_equal, fill=1.0, base=0,
                             pattern=[[-1, 9]], channel_multiplier=1)
            gp.nop().then_inc(s_eye, 1)

        @block.scalar
        def _(sc):
            # second HW DGE ring
            sc.dma_start(out=X[:, 2730:ALEN],
                         in_=xa[0][:, 2730:ALEN]).then_inc(s_in2, 16)
            sc.dma_start(out=X[:, ALEN:SH], in_=xb[0]).then_inc(s_in2, 16)
            sc.dma_start(out=X[:, SH + ALEN:2 * SH],
                         in_=xb[1]).then_inc(s_in2, 16)
            # dummy sqrt to trigger the ACT table load early
            sc.activation(ST[:, 7:8], ST[:, 6:7], ActF.Sqrt)
            sc.wait_ge(s_in1, 16)
            sc.activation(SQ[:, :], SGB[:, 0:1024], ActF.Square,
                          accum_out=ST[:, 1:2]).then_inc(s_sq, 1)
            sc.wait_ge(s_bn1, 1)
            # ST7 = sqrt(-S1^2 + ST3); self-sem hop commits the write
            sc.activation(ST[:, 7:8], ST[:, 2:3], ActF.Sqrt, scale=-1.0,
                          bias=ST[:, 3:4]).then_inc(s_a, 1)
            sc.wait_ge(s_a, 1)
            sc.nop().then_inc(s_act1, 1)

            na = [1]

            def evac(pt_ranges, oa_off, n_chunks, g):
                sc.wait_ge(s_pe, g + 1)
                off = oa_off
                for lo, hi in pt_ranges:
                    sc.activation(OA[:, off: off + (hi - lo)],
                                  PT[:, lo:hi], ActF.Copy).then_inc(s_a, 1)
                    off += hi - lo
                    na[0] += 1
                # commit hop so the out-DMA reads a completed OA write tail
                sc.wait_ge(s_a, na[0])
                sc.nop().then_inc(s_act, n_chunks)

            def product(cb, di, dj, dst, init=False):
                tap = xcb_view(cb)[:, R_PE + di: R_PE + di + R_DVE,
                                   dj: dj + 128]
                sc.activation(dst.rearrange("p (r c) -> p r c", c=128), tap,
                              ActF.Copy,
                              scale=WCH[:, cb * 9 + 3 * di + dj:
                                        cb * 9 + 3 * di + dj + 1]
                              ).then_inc(s_ta, 1)

            # cb0 tap products into TA
            sc.wait_ge(s_wc, 1)
            sc.wait_ge(s_in1, 32)
            sc.wait_ge(s_in2, 32)
            for j, (di, dj) in enumerate(ACT_TAPS):
                product(0, di, dj, TA[:, j * RD:(j + 1) * RD])
            # cb1 DIAG builds (DVE builds cb0's)
            sc.wait_ge(s_eye, 2)
            for p in range(9):
                i = 9 + p
                sc.activation(DIAG[:, i * 128:(i + 1) * 128], EYE128[:, :],
                              ActF.Copy,
                              scale=WCH[:, i:i + 1]).then_inc(s_wch2, 1)
            evac([(0, 2048)], 0, 4, 0)           # A0: banks 0-3
            evac([(2048, 4096)], 2048, 4, 1)     # A1: banks 4-7
            evac([(0, 1024)], 4096, 2, 2)        # A2: banks 0-1
            # cb1 tap products
            sc.wait_ge(s_in1, 48)
            sc.wait_ge(s_in2, 48)
            for j, (di, dj) in enumerate(ACT_TAPS):
                product(1, di, dj, TA[:, (4 + j) * RD:(5 + j) * RD])
            evac([(1024, 3072)], 8192, 4, 3)     # B0: banks 2-5
            sc.dma_start(out=out1[:, 0:2048],
                         in_=OA[:, 8192:10240]).then_inc(s_out, 16)
            evac([(3072, 4096), (0, 1024)], 10240, 4, 4)  # B1: banks 6701
            sc.dma_start(out=out1[:, 2048:4096],
                         in_=OA[:, 10240:12288]).then_inc(s_out, 16)
            evac([(1024, 1536)], 12288, 1, 5)    # B2a: bank 2 (chunk 8)
            evac([(1536, 2048)], 12800, 1, 6)    # B2b: bank 3 (chunk 9)
            sc.dma_start(out=out1[:, 4096:5120],
                         in_=OA[:, 12288:13312]).then_inc(s_out, 16)

        @block.vector
        def _(ve):
            # back-to-back dependent DVE ops on tiny operands race (the next
            # op's reads overlap the previous op's in-flight writes), so the
            # whole BN small-op chain is self-semaphored hop by hop.
            vc = [0]

            def step(ins):
                vc[0] += 1
                ins.then_inc(s_v, 1)
                ve.wait_ge(s_v, vc[0])

            # BN statistics in the raw-sums basis, minimal serial depth:
            #   Dv = 1024*S2 - S1^2 = 1024^2 * var_raw
            #   s  = 1024*gamma / sqrt(Dv + 1024^2*eps)
            #   t  = beta - (S1/1024)*s
            ve.wait_ge(s_in1, 16)
            ve.tensor_scalar(ST[:, 10:11], SGB[:, 1024:1025], 1024.0, None,
                             Alu.mult)                         # gamma*1024
            step(ve.tensor_reduce(ST[:, 0:1], SGB[:, 0:1024],
                                  axis=mybir.AxisListType.X, op=Alu.add))
            ve.tensor_scalar(ST[:, 2:3], ST[:, 0:1], ST[:, 0:1], None,
                             Alu.mult)                         # S1^2
            ve.tensor_scalar(ST[:, 4:5], ST[:, 0:1], -1.0 / 1024, None,
                             Alu.mult)                         # -S1/1024
            ve.wait_ge(s_sq, 1)   # ACT's square also accumulated S2
            step(ve.tensor_scalar(ST[:, 3:4], ST[:, 1:2], 1024.0,
                                  1048576.0 * EPS_EFF, Alu.mult, Alu.add))
            ve.nop().then_inc(s_bn1, 1)
            # ACT computes ST7 = sqrt(ST3 - S1^2) with the subtract fused
            ve.wait_ge(s_act1, 1)
            step(ve.reciprocal(ST[:, 8:9], ST[:, 7:8]))
            step(ve.tensor_scalar(ST[:, 9:10], ST[:, 10:11], ST[:, 8:9],
                                  None, Alu.mult))             # s
            step(ve.scalar_tensor_tensor(ST[:, 11:12], ST[:, 4:5],
                                         ST[:, 9:10], SGB[:, 1025:1026],
                                         Alu.mult, Alu.add))   # t
            step(ve.tensor_scalar(WSM[:, :], SGB[:, 0:256], ST[:, 9:10],
                                  ST[:, 11:12], Alu.mult, Alu.add))
            ve.nop().then_inc(s_bn2, 1)
            ve.wait_ge(s_tp, 1)
            ve.tensor_copy(WCH[:, 0:9], PT[:, 0:9])
            step(ve.tensor_copy(WCH[:, 9:18], PT[:, 9:18]))
            ve.nop().then_inc(s_wc, 1)
            ve.wait_ge(s_eye, 2)
            for p in range(9):  # cb0 DIAGs (ACT builds cb1's)
                ve.tensor_scalar(DIAG[:, p * 128:(p + 1) * 128],
                                 EYE128[:, :], WCH[:, p:p + 1], None,
                                 Alu.mult).then_inc(s_wch, 1)
            # DVE stencil region: rows [R_PE, 64) of each channel block
            for cb in (0, 1):
                accv = ACC[:, :].rearrange("p (r c) -> p r c", c=128)
                tmpv = TMP[:, :].rearrange("p (r c) -> p r c", c=128)
                outv = OA[:, cb * 8192 + R_PE * 128: cb * 8192 + 8192]
                outv = outv.rearrange("p (r c) -> p r c", c=128)
                xcb = xcb_view(cb)
                if cb == 0:
                    ve.wait_ge(s_in1, 32)
                    ve.wait_ge(s_in2, 32)
                else:
                    ve.wait_ge(s_in1, 48)
                    ve.wait_ge(s_in2, 48)
                taps = [(0, 2)] + DVE_TAPS  # first tap initializes ACC
                for i, (di, dj) in enumerate(taps):
                    tap = xcb[:, R_PE + di: R_PE + di + R_DVE, dj: dj + 128]
                    wsc = WCH[:, cb * 9 + 3 * di + dj:
                              cb * 9 + 3 * di + dj + 1]
                    if i == 0:
                        ve.tensor_scalar(accv, tap, wsc, None, Alu.mult)
                    else:
                        ve.tensor_scalar(tmpv, tap, wsc, None, Alu.mult)
                        ve.tensor_tensor(accv, tmpv, accv, Alu.add)
                for j in range(4):  # ACT-produced tap products
                    ve.wait_ge(s_ta, 4 * cb + j + 1)
                    tav = TA[:, (4 * cb + j) * RD: (4 * cb + j + 1) * RD]
                    tav = tav.rearrange("p (r c) -> p r c", c=128)
                    if j < 3:
                        last = ve.tensor_tensor(accv, tav, accv, Alu.add)
                    else:
                        # final tap writes OA in halves so the drain of the
                        # first half overlaps the second
                        h = R_DVE // 2
                        step(ve.tensor_tensor(outv[:, 0:h], tav[:, 0:h],
                                              accv[:, 0:h], Alu.add))
                        ve.nop().then_inc(s_dve, 1)
                        last = ve.tensor_tensor(outv[:, h:R_DVE],
                                                tav[:, h:R_DVE],
                                                accv[:, h:R_DVE], Alu.add)
                # self-sem hop so the OA write tail is committed before
                # the out-DMA (sem'd via s_dve) reads it
                step(last)
                ve.nop().then_inc(s_dve, 1)

        @block.tensor
        def _(te):
            te.wait_ge(s_eye, 2)
            # HAM warm-up bursts: keep PE busy (never >3.4us idle) through
            # the BN-weights chain without blocking real work for long
            for _ in range(11):
                te.matmul(PT[:, 3584:4096], lhsT=EYE128[:, :],
                          rhs=X[:, 0:512], start=True, stop=True)
            te.wait_ge(s_bn2, 1)
            # both transposes land in psum bank 0: the second must not
            # re-clear the bank (start=True wipes the whole bank)
            te.matmul(PT[:, 0:9], lhsT=WSM[:, 0:128], rhs=EYE9[:, :],
                      is_transpose=True, start=True, stop=False,
                      skip_group_check=True)
            te.matmul(PT[:, 9:18], lhsT=WSM[:, 128:256], rhs=EYE9[:, :],
                      is_transpose=True, start=False, stop=True,
                      skip_group_check=True).then_inc(s_tp, 1)
            for _ in range(2):  # stay warm through the diag builds
                te.matmul(PT[:, 3584:4096], lhsT=EYE128[:, :],
                          rhs=X[:, 0:512], start=True, stop=True)
            te.wait_ge(s_in1, 32)  # xa0 loaded
            te.wait_ge(s_in2, 16)
            for g, (cb, grp) in enumerate(GROUPS):
                if cb == 1 and grp[0] == 0:
                    te.wait_ge(s_in1, 48)   # xa1 loaded
                    te.wait_ge(s_wch2, 9)   # cb1 DIAGs built
                if ACT_WAIT[g]:
                    te.wait_ge(s_act, ACT_WAIT[g])
                xcb = xcb_view(cb)
                for p in range(9):
                    if g == 0:
                        te.wait_ge(s_wch, p + 1)
                    di, dj = p // 3, p % 3
                    lhsT = DIAG[:, (cb * 9 + p) * 128:
                                (cb * 9 + p) * 128 + 128]
                    for c in grp:
                        rhs = xcb[:, 4 * c + di: 4 * c + di + 4,
                                  dj: dj + 128]
                        mm = te.matmul(
                            PT[:, BANK[cb][c] * 512:
                               BANK[cb][c] * 512 + 512],
                            lhsT=lhsT, rhs=rhs,
                            start=(p == 0), stop=(p == 8))
                mm.then_inc(s_pe, 1)

    return nc


def host_prep(x, w_conv):
    """Shard + pack all phase inputs (layout only, no math)."""
    n = x.shape[0]
    xpad = np.zeros((n, C, H + 2, W + 2), np.float32)
    xpad[:, :, 1:-1, 1:-1] = x
    xbf = xpad.astype(bf16)
    wt = np.ascontiguousarray(
        w_conv.reshape(9, 2, 128).transpose(2, 1, 0).reshape(128, 18)
    ).astype(bf16)
    maps_a, maps_b = [], []
    for k in range(N_CORES):
        sh = xbf[k // 2, :, 64 * (k % 2):64 * (k % 2) + 66, :]  # (256,66,130)
        shv = np.ascontiguousarray(sh).reshape(2, 128, RS, CS)
        # phase A: interior rows 1..64, cols 1..128, chunk-major (half, cb)
        xi = shv[:, :, 1:65, 1:129]                 # (2,128,64,128)
        xa_in = np.ascontiguousarray(
            xi.reshape(2, 128, 2, 32 * 128).transpose(2, 0, 1, 3)
        ).reshape(4, 128, 4096)
        x0_in = np.ascontiguousarray(
            np.concatenate([wt, xa_in[0]], axis=1))
        maps_a.append({"x0": x0_in, "x": xa_in[1:4]})
        # phase B: per-cb contiguous chunks
        flat = shv.reshape(2, 128, SH)
        xa_b = np.ascontiguousarray(flat[:, :, 0:ALEN])
        xb_b = np.ascontiguousarray(flat[:, :, ALEN:SH])
        maps_b.append({"xa": xa_b, "xb": xb_b})
    return maps_a, maps_b


def sgb_for_cores(sig, gamma, beta):
    """sig: [8, 9, 128] raw per-core sigma -> per-core sgb arrays."""
    sig_all = sig.reshape(4, 2, 9, 128).transpose(0, 2, 1, 3).reshape(4, 9, 256)
    out = []
    for k in range(N_CORES):
        ni = k // 2
        order = [ni] + [j for j in range(4) if j != ni]
        sgb = np.zeros((9, 1026), np.float32)
        sgb[:, 0:1024] = sig_all[order].transpose(1, 0, 2).reshape(9, 1024)
        sgb[:, 1024] = gamma
        sgb[:, 1025] = beta
        out.append(sgb.astype(bf16))
    return out


def assemble_output(res_b, n):
    outf = np.empty((n, C, H, W), np.float32)
    for k in range(N_CORES):
        r = res_b.results[k]
        ni, r0 = k // 2, 64 * (k % 2)
        for cb, name in enumerate(("out0", "out1")):
            chs = slice(cb * 128, cb * 128 + 128)
            outf[ni, chs, r0:r0 + 64, :] = \
                np.asarray(r[name]).reshape(128, 64, W).astype(np.float32)
    return outf


_CACHE = {}


def kernel(x, w_conv, gamma, beta):
    x = np.asarray(x, dtype=np.float32)
    w_conv = np.asarray(w_conv, dtype=np.float32)
    gamma = np.asarray(gamma, dtype=np.float32)
    beta = np.asarray(beta, dtype=np.float32)

    if "A" not in _CACHE:
        _CACHE["A"] = build_phase_a()
        _CACHE["B"] = build_phase_b()

    maps_a, maps_b = host_prep(x, w_conv)
    res_a = run_bass_kernel_spmd(_CACHE["A"], maps_a, CORE_IDS)
    sig = np.stack([np.asarray(res_a.results[k]["sig"]) for k in CORE_IDS])
    sgbs = sgb_for_cores(sig, gamma, beta)
    for m, sgb in zip(maps_b, sgbs):
        m["sgb"] = sgb
    res_b = run_bass_kernel_spmd(_CACHE["B"], maps_b, CORE_IDS)
    return assemble_output(res_b, x.shape[0])
